# revision 77
# baseline (speedup 1.0000x reference)
"""Trainium2 Bass kernel for the DPLR state-space model (S4-style FFT conv).

Strategy (no collectives; 8 cores = 4 batches x 2 channel-halves):
  - Host precomputes (fp64-exact) the chunked-SSM operators from the tiny
    [D,N] SSM parameters: Toeplitz intra-chunk conv matrices S_T (with the
    skip connection folded into lag 0), state-gather weights W_state, the
    state-broadcast matrices Qp, and chunk-decay factors A^T.
  - Device per core (b = core//2, h = core%2, 512 local channels):
      proj1: x_in = x[b] @ W_in[half]^T  (fp16 matmuls, chunk layout)
      conv:  per-channel intra-chunk matmul (V_d stationary [128,16],
             Toeplitz moving [128,128]) + chunk-state recurrence via
             tensor_tensor_scan on DVE + inter-chunk correction matmuls
             (K=16, accumulated into the same PSUM strips)
      proj2: partial out_T = W_out[:, half-cols] @ y   (fp16)
  - Host sums the two partial outputs per batch (exact linearity of W_out).

Chunking: L=2048 split into C=16 chunks of T=128.  All matmul operands are
fp16 (PSUM accumulation in fp32); expected end-to-end relative error ~1e-3.
"""

import numpy as np

import concourse.bass as bass
import concourse.bacc as bacc
import concourse.mybir as mybir
from concourse.tile import TileContext
from concourse.bass_utils import run_bass_kernel_spmd

# Problem shape (hardcoded per contract)
B, L, D, N = 4, 2048, 1024, 16
T = 128          # chunk length == SBUF partitions
CN = L // T      # 16 chunks
DL = D // 2      # 512 local channels per core
NG = DL // 4     # 128 groups of 4 channels (one PSUM bank strip set each)

DT = mybir.dt.float16
F32 = mybir.dt.float32


# --------------------------------------------------------------------------
# Device program (identical on all 8 cores; SPMD over per-core data)
# --------------------------------------------------------------------------

def build_nc(sim_safe=False):
    # sim_safe=True adds PSUM-pad memsets that CoreSim's uninitialized-read
    # shadow checker requires; the pads never reach valid outputs, so the
    # shipped kernel omits them.
    nc = bacc.Bacc()

    xT = nc.declare_dram_parameter("xT", [128, 8 * 2048], DT, isOutput=False)
    winT = nc.declare_dram_parameter("winT", [128, 8 * 512], DT, isOutput=False)
    woutT = nc.declare_dram_parameter("woutT", [128, 4 * 1024], DT, isOutput=False)
    s_t = nc.declare_dram_parameter("s_t", [128, DL * 128], DT, isOutput=False)
    wstate = nc.declare_dram_parameter("wstate", [128, DL * 16], DT, isOutput=False)
    qp = nc.declare_dram_parameter("qp", [128, NG * 128], DT, isOutput=False)
    d0 = nc.declare_dram_parameter("d0", [128, NG * 32], DT, isOutput=False)
    outT = nc.declare_dram_parameter("outT", [128, 8 * 2048], DT, isOutput=True)
    # DRAM bounce buffer for the conv-output partition regroup:
    # yscr[dl, c*128 + t] = y[dl, l],  dl = 4k + s (group-major)
    yscr = nc.dram_tensor("yscr", [DL, 2048], DT)

    with TileContext(nc) as tc:
        with (
            tc.tile_pool(name="cpool", bufs=1) as cpool,
        ):
            # persistent SBUF tensors (straight contiguous loads).
            # DMA ring split: big operator loads on SWDGE (gpsimd), the S_T
            # stream on ACT's HWDGE ring, regroup/output on SP's ring.
            # tiles for operator loads; DMAs are emitted at their need-time
            # (emission order ~ scheduler priority ~ global-DMA-device order)
            woutT_sb = cpool.tile([128, 4 * 1024], DT, name="woutT_sb")
            wstate_sb = cpool.tile([128, DL * 16], DT, name="wstate_sb")
            qp_sb = cpool.tile([128, NG * 128], DT, name="qp_sb")
            d0_sb = cpool.tile([128, NG * 32], DT, name="d0_sb")


            # x_in in chunk layout: xall[j, dl*16 + c] (dl-major so each
            # channel's V_d = xall[:, dl*16:(dl+1)*16] is a contiguous slice
            # with channel-granular dependency tracking)
            xall = cpool.tile([128, CN * DL], DT, name="xall")
            # chunk-state scan buffers: partition 32*s + n, free k*32 + q
            scanbuf = cpool.tile([128, NG * 32], DT, name="scanbuf")
            scanout = cpool.tile([128, NG * 32], DT, name="scanout")
            # conv output: y_sb[k, s*2048 + c*128 + t] = y[dl = s*128+k, l]
            y_sb = cpool.tile([128, 4 * 2048], DT, name="y_sb")

            nc.vector.memset(scanbuf[:, :], 0.0)

            # proj1-copy view: [j, c, dl] (strided dst, c = l-tile)
            xall_c = xall.rearrange("p (d c) -> p c d", c=CN)

            # S_T stream pool allocated BEFORE the proj1 pool so its SBUF
            # range doesn't reuse proj1's (a stack-reuse dependency would
            # delay the S_T stream until proj1 finishes)
            spool = tc.alloc_tile_pool(name="spool", bufs=4)

            # ---- proj1: x_in[l, dl] = sum_f x[l,f] W_in[dl,f]
            # one PSUM pool for proj1 + pass A (separate tags, 4+4 banks) so
            # pass A overlaps proj1's second half instead of waiting on a
            # bank-reuse dependency
            with (
                tc.tile_pool(name="xpool", bufs=1) as xpool,
                tc.tile_pool(name="pp1", bufs=3, space="PSUM") as pp1,
                tc.tile_pool(name="ppA", bufs=5, space="PSUM") as ppA,
            ):
                # proj1 inputs on the SP ring (idle early) so PE starts ASAP;
                # xT split into two DMAs so the first l-half unblocks sooner
                xT_sb = xpool.tile([128, 8 * 2048], DT, name="xT_sb")
                winT_sb = xpool.tile([128, 8 * 512], DT, name="winT_sb")
                nc.scalar.dma_start(out=winT_sb[:, :], in_=winT[:, :])
                xt3 = xT_sb.rearrange("p (ft l) -> p ft l", l=2048)
                xd3 = xT.rearrange("p (ft l) -> p ft l", l=2048)
                for xq in range(4):
                    nc.sync.dma_start(out=xt3[:, :, xq * 512:(xq + 1) * 512],
                                      in_=xd3[:, :, xq * 512:(xq + 1) * 512])
                # needed from pass A / the scan onward — same HWDGE ring as
                # x/W so per-engine FIFO order keeps them behind proj1's data
                nc.sync.dma_start(out=wstate_sb[:, :], in_=wstate[:, :])
                nc.sync.dma_start(out=d0_sb[:, :], in_=d0[:, :])
                def proj1_tile(dh, lt):
                    ps1 = pp1.tile([128, 256], F32, tag="ps1", name="ps1")
                    for ft in range(8):
                        nc.tensor.matmul(
                            ps1[:, :],
                            lhsT=xT_sb[:, ft * 2048 + lt * 128: ft * 2048 + (lt + 1) * 128],
                            rhs=winT_sb[:, ft * 512 + dh * 256: ft * 512 + (dh + 1) * 256],
                            start=(ft == 0), stop=(ft == 7),
                        )
                    nc.vector.tensor_copy(
                        xall_c[:, lt, dh * 256:(dh + 1) * 256], ps1[:, :])

                def passA_group(k):
                    # u[n, c] for the 4 channels of group k -> scanbuf
                    # padded so each tile owns whole 2KB PSUM zero-regions
                    # (start=True clears a full region; sharing one with an
                    # open accumulation group would corrupt it)
                    psA = ppA.tile([128, 16], F32, tag="psA", name="psA",
                                   padded_shape=[128, 512])
                    if sim_safe:
                        nc.vector.memset(psA[:, :], 0.0)
                    for s in range(4):
                        dl = 4 * k + s         # group-major channel mapping
                        nc.tensor.matmul(
                            psA[32 * s:32 * s + 16, :],
                            lhsT=wstate_sb[:, dl * 16:(dl + 1) * 16],
                            rhs=xall[:, dl * 16:(dl + 1) * 16],
                            start=True, stop=True,
                            tile_position=(0, 32 * s),
                        )
                    # ACT copy: frees DVE for the scan chunks and xall copies
                    nc.scalar.copy(
                        scanbuf[:, k * 32 + 1:k * 32 + 17], psA[:, 0:16])

                def scan_chunk(sc):
                    # chunk-state scan: P[c] = aT*P[c-1] + u[c-1], reset per
                    # 32-slot group window; covers groups [32sc, 32sc+32)
                    sl_ = slice(sc * NG * 8, (sc + 1) * NG * 8)
                    nc.vector.tensor_tensor_scan(
                        out=scanout[:, sl_], data0=d0_sb[:, sl_],
                        data1=scanbuf[:, sl_],
                        initial=0.0, op0=mybir.AluOpType.mult,
                        op1=mybir.AluOpType.add,
                    )

                for lt in range(CN):
                    proj1_tile(0, lt)
                # qp is first needed by pass B's inter matmuls
                nc.gpsimd.dma_start(out=qp_sb[:, :], in_=qp[:, :])
                # second dl-half of proj1 interleaved with pass A for the
                # first dl-half's groups (k < 64): keeps PE dense and spreads
                # the DVE psA copies under proj1's matmuls
                for lt in range(CN):
                    proj1_tile(1, lt)
                    for k in range(4 * lt, 4 * lt + 4):
                        passA_group(k)
                    if lt == 7:
                        scan_chunk(0)
                scan_chunk(1)
                for k in range(64, NG):
                    passA_group(k)
                    if k == 95:
                        scan_chunk(2)
                scan_chunk(3)

            # woutT is first needed by proj2 (late); load it mid-stream
            nc.gpsimd.dma_start(out=woutT_sb[:, :], in_=woutT[:, :])

            # ---- pass B: y = S_T^T V (intra) + Qp^T P (inter)
            # shares one PSUM pool with proj2 (tags psB=5 / ps2=3 banks) so
            # proj2's per-d-tile accumulation overlaps the pass B tail
            with (
                tc.tile_pool(name="ypool", bufs=3) as ypool,
                tc.tile_pool(name="ppB", bufs=5, space="PSUM") as ppB,
                tc.tile_pool(name="pp2", bufs=3, space="PSUM") as pp2,
            ):
                for kbb in range(4):           # super-block: 32 groups/128 ch
                    yst = ypool.tile([128, 32 * 128], DT, tag="yst", name="yst")
                    for q in range(4):         # s_t stream blocks of 8 groups
                        qg = kbb * 4 + q
                        s_blk = spool.tile([128, 32 * 128], DT, tag="s_blk",
                                           name="s_blk")
                        nc.scalar.dma_start(
                            out=s_blk[:, :], in_=s_t[:, qg * 4096:(qg + 1) * 4096])
                        for kk in range(8):
                            k = qg * 8 + kk
                            kks = q * 8 + kk   # slot within super-block
                            psB = ppB.tile([128, 128], F32, tag="psB", name="psB")
                            if sim_safe:
                                nc.vector.memset(psB[:, :], 0.0)
                            for s in range(4):
                                eb = kk * 4 + s    # channel index within block
                                dl = 4 * k + s     # group-major channel mapping
                                nc.tensor.matmul(
                                    psB[32 * s:32 * s + 16, :],
                                    lhsT=xall[:, dl * 16:(dl + 1) * 16],
                                    rhs=s_blk[:, eb * 128:(eb + 1) * 128],
                                    start=True, stop=False,
                                    tile_position=(0, 32 * s),
                                )
                                nc.tensor.matmul(
                                    psB[32 * s:32 * s + 16, :],
                                    lhsT=scanout[32 * s:32 * s + 16, k * 32:k * 32 + 16],
                                    rhs=qp_sb[32 * s:32 * s + 16, k * 128:(k + 1) * 128],
                                    start=False, stop=True,
                                    tile_position=(32 * s, 32 * s),
                                )
                            if kk % 2 == 0:
                                nc.vector.tensor_copy(
                                    yst[:, kks * 128:(kks + 1) * 128], psB[:, :])
                            else:
                                nc.scalar.copy(
                                    yst[:, kks * 128:(kks + 1) * 128], psB[:, :])
                        # regroup to DRAM at half-super-block granularity,
                        # except the last super-block (quarter granularity so
                        # the final gather fires as soon as possible):
                        # yst[(32s+c), kks*128+t] -> yscr rows dl=4k+s
                        # (SBUF APs must be partition-major; DMAs split
                        #  across the SP and SWDGE rings)
                        fine = (kbb == 3)
                        if fine or q % 2 == 1:
                            qspan = 8 if fine else 16
                            g0 = (q if fine else (q // 2) * 2) * 8
                            rings = ([nc.sync, nc.gpsimd, nc.scalar, nc.sync]
                                     if fine else
                                     [nc.sync, nc.gpsimd, nc.sync, nc.gpsimd])
                            for s in range(4):
                                src = yst[32 * s:32 * s + 16,
                                          g0 * 128:(g0 + qspan) * 128].rearrange(
                                    "c (kk t) -> c kk t", t=128)
                                r0 = kbb * 128 + g0 * 4 + s
                                dst = yscr[r0: r0 + (qspan - 1) * 4 + 1: 4, :].rearrange(
                                    "kk (c t) -> c kk t", t=128)
                                rings[s].dma_start(out=dst, in_=src)
                        if fine and q == 1:
                            # first half of the last d-tile, gathered early
                            nc.sync.dma_start(
                                out=y_sb[0:64, kbb * 2048:(kbb + 1) * 2048],
                                in_=yscr[kbb * 128:kbb * 128 + 64, :])
                    # gather the finished d-tile (128 channels) back into
                    # SBUF so proj2's k-accumulation starts early
                    if kbb < 3:
                        nc.sync.dma_start(
                            out=y_sb[:, kbb * 2048:(kbb + 1) * 2048],
                            in_=yscr[kbb * 128:(kbb + 1) * 128, :])
                    else:
                        nc.sync.dma_start(
                            out=y_sb[64:128, kbb * 2048:(kbb + 1) * 2048],
                            in_=yscr[kbb * 128 + 64:(kbb + 1) * 128, :])

                # ---- proj2: out_T[e, l] = sum_dl W_out[e,dl] y[dl,l] (partial)
                opool = tc.alloc_tile_pool(name="opool", bufs=2)
                for m in range(8):
                    ost = opool.tile([128, 2048], DT, tag="ost", name="ost")
                    for lc in range(4):
                        ps2 = pp2.tile([128, 512], F32, tag="ps2", name="ps2")
                        for kt in range(4):
                            nc.tensor.matmul(
                                ps2[:, :],
                                lhsT=woutT_sb[:, kt * 1024 + m * 128: kt * 1024 + (m + 1) * 128],
                                rhs=y_sb[:, kt * 2048 + lc * 512: kt * 2048 + (lc + 1) * 512],
                                start=(kt == 0), stop=(kt == 3),
                            )
                        if lc % 2 == 0:
                            nc.vector.tensor_copy(
                                ost[:, lc * 512:(lc + 1) * 512], ps2[:, :])
                        else:
                            nc.scalar.copy(
                                ost[:, lc * 512:(lc + 1) * 512], ps2[:, :])
                    nc.sync.dma_start(
                        out=outT[:, m * 2048:(m + 1) * 2048], in_=ost[:, :])
                opool.release()

            spool.release()

    nc.finalize()
    return nc


# --------------------------------------------------------------------------
# Host-side operator precompute (fp64-exact) and data formatting
# --------------------------------------------------------------------------

def _ssm_operators(A_log, B_ssm, C_ssm, dt_log, D_ssm):
    """Full-D chunked-SSM operators, fp64."""
    A_log = A_log.astype(np.float64)
    B_ssm = B_ssm.astype(np.float64)
    C_ssm = C_ssm.astype(np.float64)
    dt_log = dt_log.astype(np.float64)
    D_ssm = D_ssm.astype(np.float64)

    A_diag = -np.exp(A_log)                       # [D, N]
    dt = np.exp(dt_log)[:, None]                  # [D, 1]
    logA = dt * A_diag                            # log(A_bar), exact
    A_bar = np.exp(logA)
    B_bar = (A_bar - 1.0) / A_diag * B_ssm
    CB = C_ssm * B_bar                            # [D, N]

    m = np.arange(T)
    A_pow = np.exp(logA[:, None, :] * m[None, :, None])       # [D, T, N]
    K = np.einsum("dn,dmn->dm", CB, A_pow)                    # [D, T]
    K[:, 0] += D_ssm                              # skip connection at lag 0

    # S_T[d, j, t] = K[d, t-j] for t >= j else 0
    idx = m[None, :] - m[:, None]                 # [j, t]
    Kp = np.concatenate([np.zeros((D, T)), K], axis=1)
    S_T = Kp[:, idx + T]                          # [D, T, T]

    W_state = np.exp(logA[:, None, :] * (T - 1 - m)[None, :, None])   # [D, T, N]
    Qp = CB[:, :, None] * np.exp(logA[:, :, None] * (m + 1)[None, None, :])  # [D,N,T]
    aT = np.exp(logA * T)                         # [D, N]
    return S_T, W_state, Qp, aT


def _half_arrays(S_T, W_state, Qp, aT, h):
    """Format one channel-half's operator arrays into device layouts (fp16).

    Channel mapping is group-major: dl = 4*k + s (stream order == dl order).
    """
    sl = slice(h * DL, (h + 1) * DL)
    S_l, W_l, Q_l, a_l = S_T[sl], W_state[sl], Qp[sl], aT[sl]

    s_t_h = np.ascontiguousarray(
        S_l.transpose(1, 0, 2).reshape(128, DL * 128)).astype(np.float16)
    wstate_h = np.ascontiguousarray(
        W_l.transpose(1, 0, 2).reshape(128, DL * 16)).astype(np.float16)

    # qp[32s+n, k*128+t] = Qp[dl=4k+s, n, t]
    q_r = Q_l.reshape(128, 4, N, T)               # [k, s, n, t]
    q_full = np.zeros((4, 32, 128, 128))
    q_full[:, :N] = q_r.transpose(1, 2, 0, 3)
    qp_h = q_full.reshape(128, NG * 128).astype(np.float16)

    # d0[32s+n, k*32+q] = aT[dl=4k+s] for q in 1..15 else 0
    a_r = a_l.reshape(128, 4, N)                  # [k, s, n]
    d0_full = np.zeros((4, 32, 128, 32))
    d0_full[:, :N, :, 1:16] = a_r.transpose(1, 2, 0)[:, :, :, None]
    d0_h = d0_full.reshape(128, NG * 32).astype(np.float16)

    return s_t_h, wstate_h, qp_h, d0_h


_NC_CACHE = None
LAST_RESULTS = None  # BassKernelResults of the most recent run (for test harness)


def _get_nc():
    global _NC_CACHE
    if _NC_CACHE is None:
        _NC_CACHE = build_nc()
    return _NC_CACHE


def prepare_in_maps(x, W_in, W_out, A_log, B_ssm, C_ssm, dt_log, D_ssm):
    x = np.asarray(x)
    W_in = np.asarray(W_in)
    W_out = np.asarray(W_out)

    S_T, W_state, Qp, aT = _ssm_operators(
        np.asarray(A_log), np.asarray(B_ssm), np.asarray(C_ssm),
        np.asarray(dt_log), np.asarray(D_ssm))

    half = [_half_arrays(S_T, W_state, Qp, aT, h) for h in range(2)]

    # per-half projection weights in device layout
    win_h, wout_h = [], []
    for h in range(2):
        Wl = W_in[h * DL:(h + 1) * DL, :]                      # [512, 1024]
        win_h.append(np.ascontiguousarray(
            Wl.T.reshape(8, 128, DL).transpose(1, 0, 2).reshape(128, 8 * DL)
        ).astype(np.float16))
        Wo = W_out[:, h * DL:(h + 1) * DL]                     # [1024, 512]
        wout_h.append(np.ascontiguousarray(
            Wo.T.reshape(4, 128, 1024).transpose(1, 0, 2).reshape(128, 4 * 1024)
        ).astype(np.float16))

    xT_b = []
    for b in range(B):
        xt = x[b].T                                            # [1024, 2048]
        xT_b.append(np.ascontiguousarray(
            xt.reshape(8, 128, L).transpose(1, 0, 2).reshape(128, 8 * L)
        ).astype(np.float16))

    in_maps = []
    for core in range(8):
        b, h = core // 2, core % 2
        s_t_h, wstate_h, qp_h, d0_h = half[h]
        in_maps.append({
            "xT": xT_b[b], "winT": win_h[h], "woutT": wout_h[h],
            "s_t": s_t_h, "wstate": wstate_h, "qp": qp_h, "d0": d0_h,
        })
    return in_maps


def run_device(in_maps):
    nc = _get_nc()
    res = run_bass_kernel_spmd(nc, in_maps, core_ids=list(range(8)))
    global LAST_RESULTS
    LAST_RESULTS = res
    return res


def gather_output(res):
    out = np.empty((B, L, D), dtype=np.float32)
    for b in range(B):
        acc = None
        for h in range(2):
            o = res.results[2 * b + h]["outT"].astype(np.float32)
            part = o.reshape(128, 8, L).transpose(1, 0, 2).reshape(D, L)
            acc = part if acc is None else acc + part
        out[b] = acc.T
    return out


def kernel(x, W_in, W_out, A_log, B_ssm, C_ssm, dt_log, D_ssm):
    in_maps = prepare_in_maps(x, W_in, W_out, A_log, B_ssm, C_ssm, dt_log, D_ssm)
    res = run_device(in_maps)
    return gather_output(res)


# revision 86
# speedup vs baseline: 2617.6448x; 2617.6448x over previous
"""Trainium2 Bass kernel for the DPLR state-space model (S4-style FFT conv).

Strategy (no collectives; 8 cores = 4 batches x 2 channel-halves):
  - Host precomputes (fp64-exact) the chunked-SSM operators from the tiny
    [D,N] SSM parameters: Toeplitz intra-chunk conv matrices S_T (with the
    skip connection folded into lag 0), state-gather weights W_state, the
    state-broadcast matrices Qp, and chunk-decay factors A^T.
  - Device per core (b = core//2, h = core%2, 512 local channels):
      proj1: x_in = x[b] @ W_in[half]^T  (fp16 matmuls, chunk layout)
      conv:  per-channel intra-chunk matmul (V_d stationary [128,16],
             Toeplitz moving [128,128]) + chunk-state recurrence via
             tensor_tensor_scan on DVE + inter-chunk correction matmuls
             (K=16, accumulated into the same PSUM strips)
      proj2: partial out_T = W_out[:, half-cols] @ y   (fp16)
  - Host sums the two partial outputs per batch (exact linearity of W_out).

Chunking: L=2048 split into C=16 chunks of T=128.  All matmul operands are
fp16 (PSUM accumulation in fp32); expected end-to-end relative error ~1e-3.
"""

import numpy as np

import concourse.bass as bass
import concourse.bacc as bacc
import concourse.mybir as mybir
from concourse.tile import TileContext
from concourse.bass_utils import run_bass_kernel_spmd

# Problem shape (hardcoded per contract)
B, L, D, N = 4, 2048, 1024, 16
T = 128          # chunk length == SBUF partitions
CN = L // T      # 16 chunks
DL = D // 2      # 512 local channels per core
NG = DL // 4     # 128 groups of 4 channels (one PSUM bank strip set each)

DT = mybir.dt.float16
F32 = mybir.dt.float32


# --------------------------------------------------------------------------
# Device program (identical on all 8 cores; SPMD over per-core data)
# --------------------------------------------------------------------------

def build_nc(sim_safe=False):
    # sim_safe=True adds PSUM-pad memsets that CoreSim's uninitialized-read
    # shadow checker requires; the pads never reach valid outputs, so the
    # shipped kernel omits them.
    nc = bacc.Bacc()

    xT = nc.declare_dram_parameter("xT", [128, 8 * 2048], DT, isOutput=False)
    winT = nc.declare_dram_parameter("winT", [128, 8 * 512], DT, isOutput=False)
    woutT = nc.declare_dram_parameter("woutT", [128, 4 * 1024], DT, isOutput=False)
    s_t = nc.declare_dram_parameter("s_t", [128, DL * 128], DT, isOutput=False)
    wstate = nc.declare_dram_parameter("wstate", [128, DL * 16], DT, isOutput=False)
    qp = nc.declare_dram_parameter("qp", [128, NG * 128], DT, isOutput=False)
    d0 = nc.declare_dram_parameter("d0", [128, NG * 32], DT, isOutput=False)
    outT = nc.declare_dram_parameter("outT", [128, 8 * 2048], DT, isOutput=True)
    # DRAM bounce buffer for the conv-output partition regroup:
    # yscr[dl, c*128 + t] = y[dl, l],  dl = 4k + s (group-major)
    yscr = nc.dram_tensor("yscr", [DL, 2048], DT)

    with TileContext(nc) as tc:
        with (
            tc.tile_pool(name="cpool", bufs=1) as cpool,
        ):
            # persistent SBUF tensors (straight contiguous loads).
            # DMA ring split: big operator loads on SWDGE (gpsimd), the S_T
            # stream on ACT's HWDGE ring, regroup/output on SP's ring.
            # tiles for operator loads; DMAs are emitted at their need-time
            # (emission order ~ scheduler priority ~ global-DMA-device order)
            woutT_sb = cpool.tile([128, 4 * 1024], DT, name="woutT_sb")
            wstate_sb = cpool.tile([128, DL * 16], DT, name="wstate_sb")
            qp_sb = cpool.tile([128, NG * 128], DT, name="qp_sb")
            d0_sb = cpool.tile([128, NG * 32], DT, name="d0_sb")


            # x_in in chunk layout: xall[j, dl*16 + c] (dl-major so each
            # channel's V_d = xall[:, dl*16:(dl+1)*16] is a contiguous slice
            # with channel-granular dependency tracking)
            xall = cpool.tile([128, CN * DL], DT, name="xall")
            # chunk-state scan buffers: partition 32*s + n, free k*32 + q
            scanbuf = cpool.tile([128, NG * 32], DT, name="scanbuf")
            scanout = cpool.tile([128, NG * 32], DT, name="scanout")
            # conv output: y_sb[k, s*2048 + c*128 + t] = y[dl = s*128+k, l]
            y_sb = cpool.tile([128, 4 * 2048], DT, name="y_sb")

            nc.vector.memset(scanbuf[:, :], 0.0)

            # proj1-copy view: [j, c, dl] (strided dst, c = l-tile)
            xall_c = xall.rearrange("p (d c) -> p c d", c=CN)

            # S_T stream pool allocated BEFORE the proj1 pool so its SBUF
            # range doesn't reuse proj1's (a stack-reuse dependency would
            # delay the S_T stream until proj1 finishes)
            spool = tc.alloc_tile_pool(name="spool", bufs=4)

            # ---- proj1: x_in[l, dl] = sum_f x[l,f] W_in[dl,f]
            # one PSUM pool for proj1 + pass A (separate tags, 4+4 banks) so
            # pass A overlaps proj1's second half instead of waiting on a
            # bank-reuse dependency
            with (
                tc.tile_pool(name="xpool", bufs=1) as xpool,
                tc.tile_pool(name="pp1", bufs=3, space="PSUM") as pp1,
                tc.tile_pool(name="ppA", bufs=5, space="PSUM") as ppA,
            ):
                # proj1 inputs on the SP ring (idle early) so PE starts ASAP;
                # xT split into two DMAs so the first l-half unblocks sooner
                xT_sb = xpool.tile([128, 8 * 2048], DT, name="xT_sb")
                winT_sb = xpool.tile([128, 8 * 512], DT, name="winT_sb")
                # winT dh=0 columns first so proj1's first psum group can
                # start as soon as xT's first quarter lands
                wt3 = winT_sb.rearrange("p (ft d) -> p ft d", d=512)
                wd3 = winT.rearrange("p (ft d) -> p ft d", d=512)
                nc.scalar.dma_start(out=wt3[:, :, 0:256], in_=wd3[:, :, 0:256])
                nc.scalar.dma_start(out=wt3[:, :, 256:512], in_=wd3[:, :, 256:512])
                xt3 = xT_sb.rearrange("p (ft l) -> p ft l", l=2048)
                xd3 = xT.rearrange("p (ft l) -> p ft l", l=2048)
                for xq in range(4):
                    nc.sync.dma_start(out=xt3[:, :, xq * 512:(xq + 1) * 512],
                                      in_=xd3[:, :, xq * 512:(xq + 1) * 512])
                # needed from pass A / the scan onward — same HWDGE ring as
                # x/W so per-engine FIFO order keeps them behind proj1's data
                nc.sync.dma_start(out=wstate_sb[:, :], in_=wstate[:, :])
                nc.sync.dma_start(out=d0_sb[:, :], in_=d0[:, :])
                def proj1_tile(dh, lt):
                    ps1 = pp1.tile([128, 256], F32, tag="ps1", name="ps1")
                    for ft in range(8):
                        nc.tensor.matmul(
                            ps1[:, :],
                            lhsT=xT_sb[:, ft * 2048 + lt * 128: ft * 2048 + (lt + 1) * 128],
                            rhs=winT_sb[:, ft * 512 + dh * 256: ft * 512 + (dh + 1) * 256],
                            start=(ft == 0), stop=(ft == 7),
                        )
                    nc.vector.tensor_copy(
                        xall_c[:, lt, dh * 256:(dh + 1) * 256], ps1[:, :])

                def passA_group(k):
                    # u[n, c] for the 4 channels of group k -> scanbuf
                    # padded so each tile owns whole 2KB PSUM zero-regions
                    # (start=True clears a full region; sharing one with an
                    # open accumulation group would corrupt it)
                    psA = ppA.tile([128, 16], F32, tag="psA", name="psA",
                                   padded_shape=[128, 512])
                    if sim_safe:
                        nc.vector.memset(psA[:, :], 0.0)
                    for s in range(4):
                        dl = 4 * k + s         # group-major channel mapping
                        nc.tensor.matmul(
                            psA[32 * s:32 * s + 16, :],
                            lhsT=wstate_sb[:, dl * 16:(dl + 1) * 16],
                            rhs=xall[:, dl * 16:(dl + 1) * 16],
                            start=True, stop=True,
                            tile_position=(0, 32 * s),
                        )
                    # ACT copy: frees DVE for the scan chunks and xall copies
                    nc.scalar.copy(
                        scanbuf[:, k * 32 + 1:k * 32 + 17], psA[:, 0:16])

                def scan_chunk(sc):
                    # chunk-state scan: P[c] = aT*P[c-1] + u[c-1], reset per
                    # 32-slot group window; covers groups [32sc, 32sc+32)
                    sl_ = slice(sc * NG * 8, (sc + 1) * NG * 8)
                    nc.vector.tensor_tensor_scan(
                        out=scanout[:, sl_], data0=d0_sb[:, sl_],
                        data1=scanbuf[:, sl_],
                        initial=0.0, op0=mybir.AluOpType.mult,
                        op1=mybir.AluOpType.add,
                    )

                for lt in range(CN):
                    proj1_tile(0, lt)
                # qp is first needed by pass B's inter matmuls
                nc.gpsimd.dma_start(out=qp_sb[:, :], in_=qp[:, :])
                # second dl-half of proj1 interleaved with pass A for the
                # first dl-half's groups (k < 64): keeps PE dense and spreads
                # the DVE psA copies under proj1's matmuls
                for lt in range(CN):
                    proj1_tile(1, lt)
                    for k in range(4 * lt, 4 * lt + 4):
                        passA_group(k)
                    if lt == 7:
                        scan_chunk(0)
                scan_chunk(1)
                for k in range(64, NG):
                    passA_group(k)
                    if k == 95:
                        scan_chunk(2)
                scan_chunk(3)

            # woutT is first needed by proj2 (late); load it mid-stream
            nc.gpsimd.dma_start(out=woutT_sb[:, :], in_=woutT[:, :])

            # ---- pass B: y = S_T^T V (intra) + Qp^T P (inter)
            # shares one PSUM pool with proj2 (tags psB=5 / ps2=3 banks) so
            # proj2's per-d-tile accumulation overlaps the pass B tail
            with (
                tc.tile_pool(name="ypool", bufs=3) as ypool,
                tc.tile_pool(name="partpool", bufs=1) as partpool,
                tc.tile_pool(name="ppB", bufs=5, space="PSUM") as ppB,
                tc.tile_pool(name="pp2", bufs=3, space="PSUM") as pp2,
            ):
                # proj2 prefold: partial sums (k-tiles 0..2) for e-blocks 0-3,
                # computed inside pass B's last super-block to fill PE bubbles
                part_sb = partpool.tile([128, 16 * 512], DT, name="part_sb")

                def prefold_unit(m, lc):
                    ps2 = pp2.tile([128, 512], F32, tag="ps2", name="ps2")
                    for kt in range(3):
                        nc.tensor.matmul(
                            ps2[:, :],
                            lhsT=woutT_sb[:, kt * 1024 + m * 128: kt * 1024 + (m + 1) * 128],
                            rhs=y_sb[:, kt * 2048 + lc * 512: kt * 2048 + (lc + 1) * 512],
                            start=(kt == 0), stop=(kt == 2),
                        )
                    u = m * 4 + lc
                    nc.vector.tensor_copy(
                        part_sb[:, u * 512:(u + 1) * 512], ps2[:, :])
                for kbb in range(4):           # super-block: 32 groups/128 ch
                    yst = ypool.tile([128, 32 * 128], DT, tag="yst", name="yst")
                    for q in range(4):         # s_t stream blocks of 8 groups
                        qg = kbb * 4 + q
                        s_blk = spool.tile([128, 32 * 128], DT, tag="s_blk",
                                           name="s_blk")
                        nc.scalar.dma_start(
                            out=s_blk[:, :], in_=s_t[:, qg * 4096:(qg + 1) * 4096])
                        for kk in range(8):
                            k = qg * 8 + kk
                            kks = q * 8 + kk   # slot within super-block
                            psB = ppB.tile([128, 128], F32, tag="psB", name="psB")
                            if sim_safe:
                                nc.vector.memset(psB[:, :], 0.0)
                            for s in range(4):
                                eb = kk * 4 + s    # channel index within block
                                dl = 4 * k + s     # group-major channel mapping
                                nc.tensor.matmul(
                                    psB[32 * s:32 * s + 16, :],
                                    lhsT=xall[:, dl * 16:(dl + 1) * 16],
                                    rhs=s_blk[:, eb * 128:(eb + 1) * 128],
                                    start=True, stop=False,
                                    tile_position=(0, 32 * s),
                                )
                                nc.tensor.matmul(
                                    psB[32 * s:32 * s + 16, :],
                                    lhsT=scanout[32 * s:32 * s + 16, k * 32:k * 32 + 16],
                                    rhs=qp_sb[32 * s:32 * s + 16, k * 128:(k + 1) * 128],
                                    start=False, stop=True,
                                    tile_position=(32 * s, 32 * s),
                                )
                            if kk % 2 == 0:
                                nc.vector.tensor_copy(
                                    yst[:, kks * 128:(kks + 1) * 128], psB[:, :])
                            else:
                                nc.scalar.copy(
                                    yst[:, kks * 128:(kks + 1) * 128], psB[:, :])
                        # regroup to DRAM at half-super-block granularity,
                        # except the last super-block (quarter granularity so
                        # the final gather fires as soon as possible):
                        # yst[(32s+c), kks*128+t] -> yscr rows dl=4k+s
                        # (SBUF APs must be partition-major; DMAs split
                        #  across the SP and SWDGE rings)
                        fine = (kbb == 3)
                        if fine or q % 2 == 1:
                            qspan = 8 if fine else 16
                            g0 = (q if fine else (q // 2) * 2) * 8
                            rings = [nc.sync, nc.gpsimd, nc.sync, nc.gpsimd]
                            for s in range(4):
                                src = yst[32 * s:32 * s + 16,
                                          g0 * 128:(g0 + qspan) * 128].rearrange(
                                    "c (kk t) -> c kk t", t=128)
                                r0 = kbb * 128 + g0 * 4 + s
                                dst = yscr[r0: r0 + (qspan - 1) * 4 + 1: 4, :].rearrange(
                                    "kk (c t) -> c kk t", t=128)
                                rings[s].dma_start(out=dst, in_=src)
                        if fine and q == 1:
                            # first half of the last d-tile, gathered early
                            nc.sync.dma_start(
                                out=y_sb[0:64, kbb * 2048:(kbb + 1) * 2048],
                                in_=yscr[kbb * 128:kbb * 128 + 64, :])
                        if fine:
                            # fill last-super-block PE bubbles with proj2
                            # partials over the already-gathered k-tiles 0..2
                            for lc in range(4):
                                prefold_unit(q, lc)
                    # gather the finished d-tile (128 channels) back into
                    # SBUF so proj2's k-accumulation starts early
                    if kbb < 3:
                        nc.sync.dma_start(
                            out=y_sb[:, kbb * 2048:(kbb + 1) * 2048],
                            in_=yscr[kbb * 128:(kbb + 1) * 128, :])
                    else:
                        nc.sync.dma_start(
                            out=y_sb[64:128, kbb * 2048:(kbb + 1) * 2048],
                            in_=yscr[kbb * 128 + 64:(kbb + 1) * 128, :])

                # ---- proj2: out_T[e, l] = sum_dl W_out[e,dl] y[dl,l] (partial)
                opool = tc.alloc_tile_pool(name="opool", bufs=2)
                for m in range(8):
                    ost = opool.tile([128, 2048], DT, tag="ost", name="ost")
                    for lc in range(4):
                        ps2 = pp2.tile([128, 512], F32, tag="ps2", name="ps2")
                        if m < 4:
                            # prefolded: only the last k-tile on PE, then add
                            # the staged partial during the copy-out
                            kt = 3
                            nc.tensor.matmul(
                                ps2[:, :],
                                lhsT=woutT_sb[:, kt * 1024 + m * 128: kt * 1024 + (m + 1) * 128],
                                rhs=y_sb[:, kt * 2048 + lc * 512: kt * 2048 + (lc + 1) * 512],
                                start=True, stop=True,
                            )
                            u = m * 4 + lc
                            nc.vector.scalar_tensor_tensor(
                                ost[:, lc * 512:(lc + 1) * 512],
                                ps2[:, :], 1.0,
                                part_sb[:, u * 512:(u + 1) * 512],
                                op0=mybir.AluOpType.mult,
                                op1=mybir.AluOpType.add,
                            )
                            continue
                        for kt in range(4):
                            nc.tensor.matmul(
                                ps2[:, :],
                                lhsT=woutT_sb[:, kt * 1024 + m * 128: kt * 1024 + (m + 1) * 128],
                                rhs=y_sb[:, kt * 2048 + lc * 512: kt * 2048 + (lc + 1) * 512],
                                start=(kt == 0), stop=(kt == 3),
                            )
                        if lc % 2 == 0:
                            nc.vector.tensor_copy(
                                ost[:, lc * 512:(lc + 1) * 512], ps2[:, :])
                        else:
                            nc.scalar.copy(
                                ost[:, lc * 512:(lc + 1) * 512], ps2[:, :])
                    nc.sync.dma_start(
                        out=outT[:, m * 2048:(m + 1) * 2048], in_=ost[:, :])
                opool.release()

            spool.release()

    nc.finalize()
    return nc


# --------------------------------------------------------------------------
# Host-side operator precompute (fp64-exact) and data formatting
# --------------------------------------------------------------------------

def _ssm_operators(A_log, B_ssm, C_ssm, dt_log, D_ssm):
    """Full-D chunked-SSM operators, fp64."""
    A_log = A_log.astype(np.float64)
    B_ssm = B_ssm.astype(np.float64)
    C_ssm = C_ssm.astype(np.float64)
    dt_log = dt_log.astype(np.float64)
    D_ssm = D_ssm.astype(np.float64)

    A_diag = -np.exp(A_log)                       # [D, N]
    dt = np.exp(dt_log)[:, None]                  # [D, 1]
    logA = dt * A_diag                            # log(A_bar), exact
    A_bar = np.exp(logA)
    B_bar = (A_bar - 1.0) / A_diag * B_ssm
    CB = C_ssm * B_bar                            # [D, N]

    m = np.arange(T)
    A_pow = np.exp(logA[:, None, :] * m[None, :, None])       # [D, T, N]
    K = np.einsum("dn,dmn->dm", CB, A_pow)                    # [D, T]
    K[:, 0] += D_ssm                              # skip connection at lag 0

    # S_T[d, j, t] = K[d, t-j] for t >= j else 0
    idx = m[None, :] - m[:, None]                 # [j, t]
    Kp = np.concatenate([np.zeros((D, T)), K], axis=1)
    S_T = Kp[:, idx + T]                          # [D, T, T]

    W_state = np.exp(logA[:, None, :] * (T - 1 - m)[None, :, None])   # [D, T, N]
    Qp = CB[:, :, None] * np.exp(logA[:, :, None] * (m + 1)[None, None, :])  # [D,N,T]
    aT = np.exp(logA * T)                         # [D, N]
    return S_T, W_state, Qp, aT


def _half_arrays(S_T, W_state, Qp, aT, h):
    """Format one channel-half's operator arrays into device layouts (fp16).

    Channel mapping is group-major: dl = 4*k + s (stream order == dl order).
    """
    sl = slice(h * DL, (h + 1) * DL)
    S_l, W_l, Q_l, a_l = S_T[sl], W_state[sl], Qp[sl], aT[sl]

    s_t_h = np.ascontiguousarray(
        S_l.transpose(1, 0, 2).reshape(128, DL * 128)).astype(np.float16)
    wstate_h = np.ascontiguousarray(
        W_l.transpose(1, 0, 2).reshape(128, DL * 16)).astype(np.float16)

    # qp[32s+n, k*128+t] = Qp[dl=4k+s, n, t]
    q_r = Q_l.reshape(128, 4, N, T)               # [k, s, n, t]
    q_full = np.zeros((4, 32, 128, 128))
    q_full[:, :N] = q_r.transpose(1, 2, 0, 3)
    qp_h = q_full.reshape(128, NG * 128).astype(np.float16)

    # d0[32s+n, k*32+q] = aT[dl=4k+s] for q in 1..15 else 0
    a_r = a_l.reshape(128, 4, N)                  # [k, s, n]
    d0_full = np.zeros((4, 32, 128, 32))
    d0_full[:, :N, :, 1:16] = a_r.transpose(1, 2, 0)[:, :, :, None]
    d0_h = d0_full.reshape(128, NG * 32).astype(np.float16)

    return s_t_h, wstate_h, qp_h, d0_h


_NC_CACHE = None
LAST_RESULTS = None  # BassKernelResults of the most recent run (for test harness)


def _get_nc():
    global _NC_CACHE
    if _NC_CACHE is None:
        _NC_CACHE = build_nc()
    return _NC_CACHE


def prepare_in_maps(x, W_in, W_out, A_log, B_ssm, C_ssm, dt_log, D_ssm):
    x = np.asarray(x)
    W_in = np.asarray(W_in)
    W_out = np.asarray(W_out)

    S_T, W_state, Qp, aT = _ssm_operators(
        np.asarray(A_log), np.asarray(B_ssm), np.asarray(C_ssm),
        np.asarray(dt_log), np.asarray(D_ssm))

    half = [_half_arrays(S_T, W_state, Qp, aT, h) for h in range(2)]

    # per-half projection weights in device layout
    win_h, wout_h = [], []
    for h in range(2):
        Wl = W_in[h * DL:(h + 1) * DL, :]                      # [512, 1024]
        win_h.append(np.ascontiguousarray(
            Wl.T.reshape(8, 128, DL).transpose(1, 0, 2).reshape(128, 8 * DL)
        ).astype(np.float16))
        Wo = W_out[:, h * DL:(h + 1) * DL]                     # [1024, 512]
        wout_h.append(np.ascontiguousarray(
            Wo.T.reshape(4, 128, 1024).transpose(1, 0, 2).reshape(128, 4 * 1024)
        ).astype(np.float16))

    xT_b = []
    for b in range(B):
        xt = x[b].T                                            # [1024, 2048]
        xT_b.append(np.ascontiguousarray(
            xt.reshape(8, 128, L).transpose(1, 0, 2).reshape(128, 8 * L)
        ).astype(np.float16))

    in_maps = []
    for core in range(8):
        b, h = core // 2, core % 2
        s_t_h, wstate_h, qp_h, d0_h = half[h]
        in_maps.append({
            "xT": xT_b[b], "winT": win_h[h], "woutT": wout_h[h],
            "s_t": s_t_h, "wstate": wstate_h, "qp": qp_h, "d0": d0_h,
        })
    return in_maps


def run_device(in_maps):
    nc = _get_nc()
    res = run_bass_kernel_spmd(nc, in_maps, core_ids=list(range(8)))
    global LAST_RESULTS
    LAST_RESULTS = res
    return res


def gather_output(res):
    out = np.empty((B, L, D), dtype=np.float32)
    for b in range(B):
        acc = None
        for h in range(2):
            o = res.results[2 * b + h]["outT"].astype(np.float32)
            part = o.reshape(128, 8, L).transpose(1, 0, 2).reshape(D, L)
            acc = part if acc is None else acc + part
        out[b] = acc.T
    return out


def kernel(x, W_in, W_out, A_log, B_ssm, C_ssm, dt_log, D_ssm):
    in_maps = prepare_in_maps(x, W_in, W_out, A_log, B_ssm, C_ssm, dt_log, D_ssm)
    res = run_device(in_maps)
    return gather_output(res)


# revision 90
# speedup vs baseline: 7942.7979x; 3.0343x over previous
"""Trainium2 Bass kernel for the DPLR state-space model (S4-style FFT conv).

Strategy (no collectives; 8 cores = 4 batches x 2 channel-halves):
  - Host precomputes (fp64-exact) the chunked-SSM operators from the tiny
    [D,N] SSM parameters: Toeplitz intra-chunk conv matrices S_T (with the
    skip connection folded into lag 0), state-gather weights W_state, the
    state-broadcast matrices Qp, and chunk-decay factors A^T.
  - Device per core (b = core//2, h = core%2, 512 local channels):
      proj1: x_in = x[b] @ W_in[half]^T  (fp16 matmuls, chunk layout)
      conv:  per-channel intra-chunk matmul (V_d stationary [128,16],
             Toeplitz moving [128,128]) + chunk-state recurrence via
             tensor_tensor_scan on DVE + inter-chunk correction matmuls
             (K=16, accumulated into the same PSUM strips)
      proj2: partial out_T = W_out[:, half-cols] @ y   (fp16)
  - Host sums the two partial outputs per batch (exact linearity of W_out).

Chunking: L=2048 split into C=16 chunks of T=128.  All matmul operands are
fp16 (PSUM accumulation in fp32); expected end-to-end relative error ~1e-3.
"""

import numpy as np

import concourse.bass as bass
import concourse.bacc as bacc
import concourse.mybir as mybir
from concourse.tile import TileContext
from concourse.bass_utils import run_bass_kernel_spmd

# Problem shape (hardcoded per contract)
B, L, D, N = 4, 2048, 1024, 16
T = 128          # chunk length == SBUF partitions
CN = L // T      # 16 chunks
DL = D // 2      # 512 local channels per core
NG = DL // 4     # 128 groups of 4 channels (one PSUM bank strip set each)

DT = mybir.dt.float16
F32 = mybir.dt.float32


# --------------------------------------------------------------------------
# Device program (identical on all 8 cores; SPMD over per-core data)
# --------------------------------------------------------------------------

def build_nc(sim_safe=False):
    # sim_safe=True adds PSUM-pad memsets that CoreSim's uninitialized-read
    # shadow checker requires; the pads never reach valid outputs, so the
    # shipped kernel omits them.
    nc = bacc.Bacc()

    xT = nc.declare_dram_parameter("xT", [128, 8 * 2048], DT, isOutput=False)
    winT = nc.declare_dram_parameter("winT", [128, 8 * 512], DT, isOutput=False)
    woutT = nc.declare_dram_parameter("woutT", [128, 4 * 1024], DT, isOutput=False)
    s_t = nc.declare_dram_parameter("s_t", [128, DL * 128], DT, isOutput=False)
    wstate = nc.declare_dram_parameter("wstate", [128, DL * 16], DT, isOutput=False)
    qp = nc.declare_dram_parameter("qp", [128, NG * 128], DT, isOutput=False)
    d0 = nc.declare_dram_parameter("d0", [128, NG * 32], DT, isOutput=False)
    outT = nc.declare_dram_parameter("outT", [128, 8 * 2048], DT, isOutput=True)
    # DRAM bounce buffer for the conv-output partition regroup:
    # yscr[dl, c*128 + t] = y[dl, l],  dl = 4k + s (group-major)
    yscr = nc.dram_tensor("yscr", [DL, 2048], DT)

    with TileContext(nc) as tc:
        with (
            tc.tile_pool(name="cpool", bufs=1) as cpool,
        ):
            # persistent SBUF tensors (straight contiguous loads).
            # DMA ring split: big operator loads on SWDGE (gpsimd), the S_T
            # stream on ACT's HWDGE ring, regroup/output on SP's ring.
            # tiles for operator loads; DMAs are emitted at their need-time
            # (emission order ~ scheduler priority ~ global-DMA-device order)
            woutT_sb = cpool.tile([128, 4 * 1024], DT, name="woutT_sb")
            wstate_sb = cpool.tile([128, DL * 16], DT, name="wstate_sb")
            qp_sb = cpool.tile([128, NG * 128], DT, name="qp_sb")
            d0_sb = cpool.tile([128, NG * 32], DT, name="d0_sb")


            # x_in in chunk layout: xall[j, dl*16 + c] (dl-major so each
            # channel's V_d = xall[:, dl*16:(dl+1)*16] is a contiguous slice
            # with channel-granular dependency tracking)
            xall = cpool.tile([128, CN * DL], DT, name="xall")
            # chunk-state scan buffers: partition 32*s + n, free k*32 + q
            scanbuf = cpool.tile([128, NG * 32], DT, name="scanbuf")
            scanout = cpool.tile([128, NG * 32], DT, name="scanout")
            # conv output: y_sb[k, s*2048 + c*128 + t] = y[dl = s*128+k, l]
            y_sb = cpool.tile([128, 4 * 2048], DT, name="y_sb")

            nc.vector.memset(scanbuf[:, :], 0.0)

            # proj1-copy view: [j, c, dl] (strided dst, c = l-tile)
            xall_c = xall.rearrange("p (d c) -> p c d", c=CN)

            # S_T stream pool allocated BEFORE the proj1 pool so its SBUF
            # range doesn't reuse proj1's (a stack-reuse dependency would
            # delay the S_T stream until proj1 finishes)
            spool = tc.alloc_tile_pool(name="spool", bufs=4)

            # ---- proj1: x_in[l, dl] = sum_f x[l,f] W_in[dl,f]
            # one PSUM pool for proj1 + pass A (separate tags, 4+4 banks) so
            # pass A overlaps proj1's second half instead of waiting on a
            # bank-reuse dependency
            with (
                tc.tile_pool(name="xpool", bufs=1) as xpool,
                tc.tile_pool(name="pp1", bufs=3, space="PSUM") as pp1,
                tc.tile_pool(name="ppA", bufs=5, space="PSUM") as ppA,
            ):
                # proj1 inputs on the SP ring (idle early) so PE starts ASAP;
                # xT split into two DMAs so the first l-half unblocks sooner
                xT_sb = xpool.tile([128, 8 * 2048], DT, name="xT_sb")
                winT_sb = xpool.tile([128, 8 * 512], DT, name="winT_sb")
                # winT dh=0 columns first so proj1's first psum group can
                # start as soon as xT's first quarter lands
                wt3 = winT_sb.rearrange("p (ft d) -> p ft d", d=512)
                wd3 = winT.rearrange("p (ft d) -> p ft d", d=512)
                nc.scalar.dma_start(out=wt3[:, :, 0:256], in_=wd3[:, :, 0:256])
                nc.scalar.dma_start(out=wt3[:, :, 256:512], in_=wd3[:, :, 256:512])
                xt3 = xT_sb.rearrange("p (ft l) -> p ft l", l=2048)
                xd3 = xT.rearrange("p (ft l) -> p ft l", l=2048)
                # first l-tile's columns alone so proj1 starts ~2us earlier
                nc.sync.dma_start(out=xt3[:, :, 0:128], in_=xd3[:, :, 0:128])
                nc.sync.dma_start(out=xt3[:, :, 128:512], in_=xd3[:, :, 128:512])
                for xq in range(1, 4):
                    nc.sync.dma_start(out=xt3[:, :, xq * 512:(xq + 1) * 512],
                                      in_=xd3[:, :, xq * 512:(xq + 1) * 512])
                # needed from pass A / the scan onward — same HWDGE ring as
                # x/W so per-engine FIFO order keeps them behind proj1's data
                nc.sync.dma_start(out=wstate_sb[:, :], in_=wstate[:, :])
                nc.sync.dma_start(out=d0_sb[:, :], in_=d0[:, :])
                def proj1_tile(dh, lt):
                    ps1 = pp1.tile([128, 256], F32, tag="ps1", name="ps1")
                    for ft in range(8):
                        nc.tensor.matmul(
                            ps1[:, :],
                            lhsT=xT_sb[:, ft * 2048 + lt * 128: ft * 2048 + (lt + 1) * 128],
                            rhs=winT_sb[:, ft * 512 + dh * 256: ft * 512 + (dh + 1) * 256],
                            start=(ft == 0), stop=(ft == 7),
                        )
                    nc.vector.tensor_copy(
                        xall_c[:, lt, dh * 256:(dh + 1) * 256], ps1[:, :])

                def passA_group(k):
                    # u[n, c] for the 4 channels of group k -> scanbuf
                    # padded so each tile owns whole 2KB PSUM zero-regions
                    # (start=True clears a full region; sharing one with an
                    # open accumulation group would corrupt it)
                    psA = ppA.tile([128, 16], F32, tag="psA", name="psA",
                                   padded_shape=[128, 512])
                    if sim_safe:
                        nc.vector.memset(psA[:, :], 0.0)
                    for s in range(4):
                        dl = 4 * k + s         # group-major channel mapping
                        nc.tensor.matmul(
                            psA[32 * s:32 * s + 16, :],
                            lhsT=wstate_sb[:, dl * 16:(dl + 1) * 16],
                            rhs=xall[:, dl * 16:(dl + 1) * 16],
                            start=True, stop=True,
                            tile_position=(0, 32 * s),
                        )
                    # ACT copy: frees DVE for the scan chunks and xall copies
                    nc.scalar.copy(
                        scanbuf[:, k * 32 + 1:k * 32 + 17], psA[:, 0:16])

                def scan_chunk(sc):
                    # chunk-state scan: P[c] = aT*P[c-1] + u[c-1], reset per
                    # 32-slot group window; covers groups [32sc, 32sc+32)
                    sl_ = slice(sc * NG * 8, (sc + 1) * NG * 8)
                    nc.vector.tensor_tensor_scan(
                        out=scanout[:, sl_], data0=d0_sb[:, sl_],
                        data1=scanbuf[:, sl_],
                        initial=0.0, op0=mybir.AluOpType.mult,
                        op1=mybir.AluOpType.add,
                    )

                for lt in range(CN):
                    proj1_tile(0, lt)
                # qp is first needed by pass B's inter matmuls
                nc.gpsimd.dma_start(out=qp_sb[:, :], in_=qp[:, :])
                # second dl-half of proj1 interleaved with pass A for the
                # first dl-half's groups (k < 64): keeps PE dense and spreads
                # the DVE psA copies under proj1's matmuls
                for lt in range(CN):
                    proj1_tile(1, lt)
                    for k in range(4 * lt, 4 * lt + 4):
                        passA_group(k)
                    if lt == 7:
                        scan_chunk(0)
                scan_chunk(1)
                for k in range(64, NG):
                    passA_group(k)
                    if k == 95:
                        scan_chunk(2)
                scan_chunk(3)

            # woutT is first needed by proj2 (late); load it mid-stream
            nc.gpsimd.dma_start(out=woutT_sb[:, :], in_=woutT[:, :])

            # ---- pass B: y = S_T^T V (intra) + Qp^T P (inter)
            # shares one PSUM pool with proj2 (tags psB=5 / ps2=3 banks) so
            # proj2's per-d-tile accumulation overlaps the pass B tail
            with (
                tc.tile_pool(name="ypool", bufs=3) as ypool,
                tc.tile_pool(name="partpool", bufs=1) as partpool,
                tc.tile_pool(name="ppB", bufs=5, space="PSUM") as ppB,
                tc.tile_pool(name="pp2", bufs=3, space="PSUM") as pp2,
            ):
                # proj2 prefold: partial sums (k-tiles 0..2) for e-blocks 0-3,
                # computed inside pass B's last super-block to fill PE bubbles
                part_sb = partpool.tile([128, 16 * 512], DT, name="part_sb")

                def prefold_unit(m, lc):
                    ps2 = pp2.tile([128, 512], F32, tag="ps2", name="ps2")
                    for kt in range(3):
                        nc.tensor.matmul(
                            ps2[:, :],
                            lhsT=woutT_sb[:, kt * 1024 + m * 128: kt * 1024 + (m + 1) * 128],
                            rhs=y_sb[:, kt * 2048 + lc * 512: kt * 2048 + (lc + 1) * 512],
                            start=(kt == 0), stop=(kt == 2),
                        )
                    u = m * 4 + lc
                    nc.vector.tensor_copy(
                        part_sb[:, u * 512:(u + 1) * 512], ps2[:, :])
                for kbb in range(4):           # super-block: 32 groups/128 ch
                    yst = ypool.tile([128, 32 * 128], DT, tag="yst", name="yst")
                    for q in range(4):         # s_t stream blocks of 8 groups
                        qg = kbb * 4 + q
                        s_blk = spool.tile([128, 32 * 128], DT, tag="s_blk",
                                           name="s_blk")
                        nc.scalar.dma_start(
                            out=s_blk[:, :], in_=s_t[:, qg * 4096:(qg + 1) * 4096])
                        for kk in range(8):
                            k = qg * 8 + kk
                            kks = q * 8 + kk   # slot within super-block
                            psB = ppB.tile([128, 128], F32, tag="psB", name="psB")
                            if sim_safe:
                                nc.vector.memset(psB[:, :], 0.0)
                            for s in range(4):
                                eb = kk * 4 + s    # channel index within block
                                dl = 4 * k + s     # group-major channel mapping
                                nc.tensor.matmul(
                                    psB[32 * s:32 * s + 16, :],
                                    lhsT=xall[:, dl * 16:(dl + 1) * 16],
                                    rhs=s_blk[:, eb * 128:(eb + 1) * 128],
                                    start=True, stop=False,
                                    tile_position=(0, 32 * s),
                                )
                                nc.tensor.matmul(
                                    psB[32 * s:32 * s + 16, :],
                                    lhsT=scanout[32 * s:32 * s + 16, k * 32:k * 32 + 16],
                                    rhs=qp_sb[32 * s:32 * s + 16, k * 128:(k + 1) * 128],
                                    start=False, stop=True,
                                    tile_position=(32 * s, 32 * s),
                                )
                            if kk % 2 == 0:
                                nc.vector.tensor_copy(
                                    yst[:, kks * 128:(kks + 1) * 128], psB[:, :])
                            else:
                                nc.scalar.copy(
                                    yst[:, kks * 128:(kks + 1) * 128], psB[:, :])
                        # regroup to DRAM at half-super-block granularity,
                        # except the last super-block (quarter granularity so
                        # the final gather fires as soon as possible):
                        # yst[(32s+c), kks*128+t] -> yscr rows dl=4k+s
                        # (SBUF APs must be partition-major; DMAs split
                        #  across the SP and SWDGE rings)
                        fine = (kbb == 3)
                        if fine or q % 2 == 1:
                            qspan = 8 if fine else 16
                            g0 = (q if fine else (q // 2) * 2) * 8
                            rings = [nc.sync, nc.gpsimd, nc.sync, nc.gpsimd]
                            for s in range(4):
                                src = yst[32 * s:32 * s + 16,
                                          g0 * 128:(g0 + qspan) * 128].rearrange(
                                    "c (kk t) -> c kk t", t=128)
                                r0 = kbb * 128 + g0 * 4 + s
                                dst = yscr[r0: r0 + (qspan - 1) * 4 + 1: 4, :].rearrange(
                                    "kk (c t) -> c kk t", t=128)
                                rings[s].dma_start(out=dst, in_=src)
                        if fine and q == 1:
                            # first half of the last d-tile, gathered early
                            nc.sync.dma_start(
                                out=y_sb[0:64, kbb * 2048:(kbb + 1) * 2048],
                                in_=yscr[kbb * 128:kbb * 128 + 64, :])
                        if fine:
                            # fill last-super-block PE bubbles with proj2
                            # partials over the already-gathered k-tiles 0..2
                            for lc in range(4):
                                prefold_unit(q, lc)
                    # gather the finished d-tile (128 channels) back into
                    # SBUF so proj2's k-accumulation starts early
                    if kbb < 3:
                        nc.sync.dma_start(
                            out=y_sb[:, kbb * 2048:(kbb + 1) * 2048],
                            in_=yscr[kbb * 128:(kbb + 1) * 128, :])
                    else:
                        nc.sync.dma_start(
                            out=y_sb[64:128, kbb * 2048:(kbb + 1) * 2048],
                            in_=yscr[kbb * 128 + 64:(kbb + 1) * 128, :])

                # ---- proj2: out_T[e, l] = sum_dl W_out[e,dl] y[dl,l] (partial)
                opool = tc.alloc_tile_pool(name="opool", bufs=2)
                for m in range(8):
                    ost = opool.tile([128, 2048], DT, tag="ost", name="ost")
                    for lc in range(4):
                        ps2 = pp2.tile([128, 512], F32, tag="ps2", name="ps2")
                        if m < 4:
                            # prefolded: only the last k-tile on PE, then add
                            # the staged partial during the copy-out
                            kt = 3
                            nc.tensor.matmul(
                                ps2[:, :],
                                lhsT=woutT_sb[:, kt * 1024 + m * 128: kt * 1024 + (m + 1) * 128],
                                rhs=y_sb[:, kt * 2048 + lc * 512: kt * 2048 + (lc + 1) * 512],
                                start=True, stop=True,
                            )
                            u = m * 4 + lc
                            nc.vector.scalar_tensor_tensor(
                                ost[:, lc * 512:(lc + 1) * 512],
                                ps2[:, :], 1.0,
                                part_sb[:, u * 512:(u + 1) * 512],
                                op0=mybir.AluOpType.mult,
                                op1=mybir.AluOpType.add,
                            )
                            continue
                        for kt in range(4):
                            nc.tensor.matmul(
                                ps2[:, :],
                                lhsT=woutT_sb[:, kt * 1024 + m * 128: kt * 1024 + (m + 1) * 128],
                                rhs=y_sb[:, kt * 2048 + lc * 512: kt * 2048 + (lc + 1) * 512],
                                start=(kt == 0), stop=(kt == 3),
                            )
                        if lc % 2 == 0:
                            nc.vector.tensor_copy(
                                ost[:, lc * 512:(lc + 1) * 512], ps2[:, :])
                        else:
                            nc.scalar.copy(
                                ost[:, lc * 512:(lc + 1) * 512], ps2[:, :])
                    if m < 7:
                        nc.sync.dma_start(
                            out=outT[:, m * 2048:(m + 1) * 2048], in_=ost[:, :])
                    else:
                        # split the last store so it drains behind the final
                        # copies instead of serializing after them
                        nc.sync.dma_start(
                            out=outT[:, m * 2048:m * 2048 + 1024],
                            in_=ost[:, 0:1024])
                        nc.sync.dma_start(
                            out=outT[:, m * 2048 + 1024:(m + 1) * 2048],
                            in_=ost[:, 1024:2048])
                opool.release()

            spool.release()

    nc.finalize()
    return nc


# --------------------------------------------------------------------------
# Host-side operator precompute (fp64-exact) and data formatting
# --------------------------------------------------------------------------

def _ssm_operators(A_log, B_ssm, C_ssm, dt_log, D_ssm):
    """Full-D chunked-SSM operators, fp64."""
    A_log = A_log.astype(np.float64)
    B_ssm = B_ssm.astype(np.float64)
    C_ssm = C_ssm.astype(np.float64)
    dt_log = dt_log.astype(np.float64)
    D_ssm = D_ssm.astype(np.float64)

    A_diag = -np.exp(A_log)                       # [D, N]
    dt = np.exp(dt_log)[:, None]                  # [D, 1]
    logA = dt * A_diag                            # log(A_bar), exact
    A_bar = np.exp(logA)
    B_bar = (A_bar - 1.0) / A_diag * B_ssm
    CB = C_ssm * B_bar                            # [D, N]

    m = np.arange(T)
    A_pow = np.exp(logA[:, None, :] * m[None, :, None])       # [D, T, N]
    K = np.einsum("dn,dmn->dm", CB, A_pow)                    # [D, T]
    K[:, 0] += D_ssm                              # skip connection at lag 0

    # S_T[d, j, t] = K[d, t-j] for t >= j else 0
    idx = m[None, :] - m[:, None]                 # [j, t]
    Kp = np.concatenate([np.zeros((D, T)), K], axis=1)
    S_T = Kp[:, idx + T]                          # [D, T, T]

    W_state = np.exp(logA[:, None, :] * (T - 1 - m)[None, :, None])   # [D, T, N]
    Qp = CB[:, :, None] * np.exp(logA[:, :, None] * (m + 1)[None, None, :])  # [D,N,T]
    aT = np.exp(logA * T)                         # [D, N]
    return S_T, W_state, Qp, aT


def _half_arrays(S_T, W_state, Qp, aT, h):
    """Format one channel-half's operator arrays into device layouts (fp16).

    Channel mapping is group-major: dl = 4*k + s (stream order == dl order).
    """
    sl = slice(h * DL, (h + 1) * DL)
    S_l, W_l, Q_l, a_l = S_T[sl], W_state[sl], Qp[sl], aT[sl]

    s_t_h = np.ascontiguousarray(
        S_l.transpose(1, 0, 2).reshape(128, DL * 128)).astype(np.float16)
    wstate_h = np.ascontiguousarray(
        W_l.transpose(1, 0, 2).reshape(128, DL * 16)).astype(np.float16)

    # qp[32s+n, k*128+t] = Qp[dl=4k+s, n, t]
    q_r = Q_l.reshape(128, 4, N, T)               # [k, s, n, t]
    q_full = np.zeros((4, 32, 128, 128))
    q_full[:, :N] = q_r.transpose(1, 2, 0, 3)
    qp_h = q_full.reshape(128, NG * 128).astype(np.float16)

    # d0[32s+n, k*32+q] = aT[dl=4k+s] for q in 1..15 else 0
    a_r = a_l.reshape(128, 4, N)                  # [k, s, n]
    d0_full = np.zeros((4, 32, 128, 32))
    d0_full[:, :N, :, 1:16] = a_r.transpose(1, 2, 0)[:, :, :, None]
    d0_h = d0_full.reshape(128, NG * 32).astype(np.float16)

    return s_t_h, wstate_h, qp_h, d0_h


_NC_CACHE = None
LAST_RESULTS = None  # BassKernelResults of the most recent run (for test harness)


def _get_nc():
    global _NC_CACHE
    if _NC_CACHE is None:
        _NC_CACHE = build_nc()
    return _NC_CACHE


def prepare_in_maps(x, W_in, W_out, A_log, B_ssm, C_ssm, dt_log, D_ssm):
    x = np.asarray(x)
    W_in = np.asarray(W_in)
    W_out = np.asarray(W_out)

    S_T, W_state, Qp, aT = _ssm_operators(
        np.asarray(A_log), np.asarray(B_ssm), np.asarray(C_ssm),
        np.asarray(dt_log), np.asarray(D_ssm))

    half = [_half_arrays(S_T, W_state, Qp, aT, h) for h in range(2)]

    # per-half projection weights in device layout
    win_h, wout_h = [], []
    for h in range(2):
        Wl = W_in[h * DL:(h + 1) * DL, :]                      # [512, 1024]
        win_h.append(np.ascontiguousarray(
            Wl.T.reshape(8, 128, DL).transpose(1, 0, 2).reshape(128, 8 * DL)
        ).astype(np.float16))
        Wo = W_out[:, h * DL:(h + 1) * DL]                     # [1024, 512]
        wout_h.append(np.ascontiguousarray(
            Wo.T.reshape(4, 128, 1024).transpose(1, 0, 2).reshape(128, 4 * 1024)
        ).astype(np.float16))

    xT_b = []
    for b in range(B):
        xt = x[b].T                                            # [1024, 2048]
        xT_b.append(np.ascontiguousarray(
            xt.reshape(8, 128, L).transpose(1, 0, 2).reshape(128, 8 * L)
        ).astype(np.float16))

    in_maps = []
    for core in range(8):
        b, h = core // 2, core % 2
        s_t_h, wstate_h, qp_h, d0_h = half[h]
        in_maps.append({
            "xT": xT_b[b], "winT": win_h[h], "woutT": wout_h[h],
            "s_t": s_t_h, "wstate": wstate_h, "qp": qp_h, "d0": d0_h,
        })
    return in_maps


def run_device(in_maps):
    nc = _get_nc()
    res = run_bass_kernel_spmd(nc, in_maps, core_ids=list(range(8)))
    global LAST_RESULTS
    LAST_RESULTS = res
    return res


def gather_output(res):
    out = np.empty((B, L, D), dtype=np.float32)
    for b in range(B):
        acc = None
        for h in range(2):
            o = res.results[2 * b + h]["outT"].astype(np.float32)
            part = o.reshape(128, 8, L).transpose(1, 0, 2).reshape(D, L)
            acc = part if acc is None else acc + part
        out[b] = acc.T
    return out


def kernel(x, W_in, W_out, A_log, B_ssm, C_ssm, dt_log, D_ssm):
    in_maps = prepare_in_maps(x, W_in, W_out, A_log, B_ssm, C_ssm, dt_log, D_ssm)
    res = run_device(in_maps)
    return gather_output(res)
